# revision 28
# baseline (speedup 1.0000x reference)
"""Trainium2 Bass kernel for prefix-attention block (B=8,T=1024,C=1024,H=16,Tp=64).

Strategy: data-parallel over batch B across 8 NeuronCores (one batch element
per core, no collectives). Per core, everything is computed in bf16 on the
TensorEngine with f32 PSUM accumulation.

v3 schedule (from trace analysis of v1 @ 317us, v2 @ 401us):
  - Head-pair software pipeline: the attention stage for pair p interleaves
    with q/k projection matmuls for pair p+1, so ScalarE exp work hides under
    projection PE work and the PE never starves long enough for the HAM clock
    gate to re-throttle.
  - Normalize/combine for pair p is emitted two pairs later so the sums
    DMA -> reciprocal -> broadcast-matmul chain never blocks the forced PE
    order (v2 stalled ~6us per pair on exactly this).
  - Warmup matmuls on the mask tile run during the ~9us DMA boot window so
    the PE hits the HAM un-throttle threshold before real work arrives.
  - kpT projection (LDWEIGHTS-bound, N=64) runs first while input DMA is
    still streaming; v-projection tail and pair-1 projections inject into
    pair-0's attention blocks.
  - qkT copies stay on ScalarE (v2 put them on DVE, which stalled score
    LDWEIGHTS); prefix sums extract immediately in front2 to free PSUM.
  - Output projection PSUM comes from the 4-slot pool with staging copies
    alternating ScalarE/VectorE; output is written bf16.

Layouts (unchanged from v1):
  qT,kT in [H*d, T] head-transposed; v natural [T, C] with a per-head ones
  column (softmax denominator falls out of the AV matmul); scores computed
  transposed sT[j,i] = k_j . q_i in [128 keys x 512 queries] PSUM tiles
  (causally trimmed at 128 granularity), exp on ScalarE (scale=1/sqrt(d)
  folded in), diagonal masked by 0/1 multiply on GpSimd, AV accumulates
  unnormalized yT + sums in PSUM; yT = A/sa + B/sb on VectorE; outT =
  w_proj^T-chunks @ yT. Host transposes the gathered output.
"""

import numpy as np
import ml_dtypes

B, T, C, H, D, TP = 8, 1024, 1024, 16, 64, 64
NT = T // 128   # 8 token tiles
KC = C // 128   # 8 contraction chunks

_CACHE = {}


def _emit(nc, tc, dram):
    import concourse.bass as bass
    import concourse.mybir as mybir
    from concourse.tile_rust import add_dep_helper
    from contextlib import ExitStack

    BF = mybir.dt.bfloat16
    F32 = mybir.dt.float32
    Exp = mybir.ActivationFunctionType.Exp

    with ExitStack() as top:
        top.enter_context(nc.allow_low_precision(
            reason="bf16 compute is intentional; f32 PSUM accumulation"))
        persist = top.enter_context(tc.tile_pool(name="persist", bufs=1))
        work = top.enter_context(tc.tile_pool(name="work", bufs=1))
        ps_gen = top.enter_context(tc.tile_pool(name="ps_gen", bufs=2, space="PSUM"))
        ps_acc = top.enter_context(tc.tile_pool(name="ps_acc", bufs=4, space="PSUM"))

        # ---- persistent SBUF tiles ----
        xT_t = [persist.tile([128, T], BF, tag=f"xT{k}", name=f"xT{k}") for k in range(KC)]
        wp_t = [persist.tile([128, C], BF, tag=f"wp{k}", name=f"wp{k}") for k in range(KC)]
        kpT = [persist.tile([128, TP], BF, tag=f"kpT{m}", name=f"kpT{m}") for m in range(8)]
        vsb = [persist.tile([128, H * 65], BF, tag=f"vsb{t}", name=f"vsb{t}") for t in range(NT)]
        vpsb = persist.tile([128, H * 65], BF, tag="vpsb", name="vpsb")
        masksb = persist.tile([128, 128], BF, tag="masksb", name="masksb")
        maskpsb = persist.tile([128, 64], BF, tag="maskpsb", name="maskpsb")
        fmat4 = persist.tile([128, 256], BF, tag="fmat4", name="fmat4")
        yT = [persist.tile([128, T], BF, tag=f"yT{t}", name=f"yT{t}") for t in range(NT)]

        wqk_pool = top.enter_context(tc.tile_pool(name="wqk_pool", bufs=24))
        qkT_q = [None] * 8
        qkT_k = [None] * 8
        pexp = top.enter_context(tc.tile_pool(name="pexp", bufs=6))
        # lead-only weights: released after the lead-in so pstg can reuse
        lead_ctx = ExitStack()
        lead = lead_ctx.enter_context(tc.tile_pool(name="lead", bufs=1))
        pT_t = [lead.tile([128, TP], BF, tag=f"pT{k}", name=f"pT{k}") for k in range(KC)]
        wkp_t = [lead.tile([128, C], BF, tag=f"wkp{k}", name=f"wkp{k}") for k in range(KC)]
        wv_t = [lead.tile([128, C], BF, tag=f"wv{k}", name=f"wv{k}") for k in range(KC)]
        wvp_t = [lead.tile([128, C], BF, tag=f"wvp{k}", name=f"wvp{k}") for k in range(KC)]

        # ---- DMA emission, in need-order ----
        nc.sync.dma_start(out=masksb, in_=dram["mask"].ap())
        for k in range(KC):
            nc.sync.dma_start(out=pT_t[k], in_=dram["pT"].ap()[k * 128:(k + 1) * 128, :])
        for k in range(KC):
            nc.sync.dma_start(out=wkp_t[k], in_=dram["wkp"].ap()[k * 128:(k + 1) * 128, :])
        for k in range(KC):
            nc.sync.dma_start(out=xT_t[k], in_=dram["xT"].ap()[k * 128:(k + 1) * 128, :])

        def dma_wqk_pair(p):
            tiles = []
            for k in range(KC):
                t = wqk_pool.tile([128, 256], BF, tag="wqk", name=f"wqk_{p}_{k}")
                nc.sync.dma_start(
                    out=t, in_=dram["wqkp"].ap()[p * C + k * 128:p * C + (k + 1) * 128, :])
                tiles.append(t)
            return tiles

        wqk0 = dma_wqk_pair(0)
        for k in range(KC):
            nc.sync.dma_start(out=wvp_t[k], in_=dram["wvp"].ap()[k * 128:(k + 1) * 128, :])
        for k in range(KC):
            nc.sync.dma_start(out=wv_t[k], in_=dram["wv"].ap()[k * 128:(k + 1) * 128, :])
        nc.sync.dma_start(out=maskpsb, in_=dram["maskp"].ap())
        nc.sync.dma_start(out=fmat4, in_=dram["fmat4"].ap())

        # ---- forced PE order ----
        pe_prev = [None]

        def pe_chain(inst):
            if pe_prev[0] is not None:
                add_dep_helper(inst.ins, pe_prev[0].ins, sync=False,
                               reason="forced PE order")
            pe_prev[0] = inst

        # ---- HAM warmup: matmuls on the mask tile while input DMA streams.
        # Results are discarded; this just keeps the PE busy past the
        # un-throttle threshold so real work starts at full clock.
        warm_ps = ps_acc.tile([128, 128], F32, tag="ps_a", name="warm_ps")
        for _ in range(40):
            pe_chain(nc.tensor.matmul(warm_ps, masksb, masksb,
                                      start=True, stop=True))

        # ---- kpT projection: LDW-bound, runs while DMA streams in ----
        for m in range(8):
            ps = ps_acc.tile([128, TP], F32, tag="ps_a", name="kp_ps")
            for k in range(KC):
                pe_chain(nc.tensor.matmul(ps, wkp_t[k][:, m * 128:(m + 1) * 128],
                                          pT_t[k], start=(k == 0), stop=(k == KC - 1)))
            nc.scalar.copy(kpT[m], ps)

        # ---- projection emitters ----
        def emit_qk_proj(p, wqk_tiles):
            qt = work.tile([128, T], BF, tag="qkTq", name=f"qkTq{p}", bufs=4)
            kt = work.tile([128, T], BF, tag="qkTk", name=f"qkTk{p}", bufs=4)
            qkT_q[p], qkT_k[p] = qt, kt
            units = []
            for which, dst in ((0, qt), (1, kt)):
                for hf in range(2):
                    units.append((which, dst, hf))
            def emit_unit(u):
                which, dst, hf = u
                po = ps_acc.tile([128, 512], F32, tag="ps_a", name="po")
                for k in range(KC):
                    pe_chain(nc.tensor.matmul(
                        po, wqk_tiles[k][:, which * 128:(which + 1) * 128],
                        xT_t[k][:, hf * 512:(hf + 1) * 512],
                        start=(k == 0), stop=(k == KC - 1)))
                if hf == 0:
                    nc.scalar.copy(dst[:, hf * 512:(hf + 1) * 512], po)
                else:
                    nc.vector.tensor_copy(dst[:, hf * 512:(hf + 1) * 512], po)
            return [(emit_unit, u) for u in units]

        def emit_v_unit(u):
            tt, hf = u
            ps = ps_gen.tile([128, 512], F32, tag="ps_g", name="ps_g")
            for k in range(KC):
                pe_chain(nc.tensor.matmul(
                    ps, xT_t[k][:, tt * 128:(tt + 1) * 128],
                    wv_t[k][:, hf * 512:(hf + 1) * 512],
                    start=(k == 0), stop=(k == KC - 1)))
            nc.vector.tensor_copy(
                vsb[tt].rearrange("p (h e) -> p h e", e=65)
                [:, hf * 8:(hf + 1) * 8, 0:64],
                ps.rearrange("p (h e) -> p h e", e=64))
            if hf == 1:
                nc.vector.memset(
                    vsb[tt].rearrange("p (h e) -> p h e", e=65)[:, :, 64:65],
                    1.0)

        # pair-0 q/k projection, then prefix v', then v tiles 0..3
        for fn, u in emit_qk_proj(0, wqk0):
            fn(u)
        for hf in range(2):
            ps = ps_gen.tile([64, 512], F32, tag="ps_g", name="ps_g")
            for k in range(KC):
                pe_chain(nc.tensor.matmul(ps, pT_t[k][:, 0:64],
                                          wvp_t[k][:, hf * 512:(hf + 1) * 512],
                                          start=(k == 0), stop=(k == KC - 1)))
            vpv = vpsb.rearrange("p (h e) -> p h e", e=65)
            nc.vector.tensor_copy(
                vpv[0:64, hf * 8:(hf + 1) * 8, 0:64],
                ps.rearrange("p (h e) -> p h e", e=64))
            nc.vector.tensor_copy(
                vpv[64:128, hf * 8:(hf + 1) * 8, 0:64],
                ps.rearrange("p (h e) -> p h e", e=64))
        nc.vector.memset(
            vpsb.rearrange("p (h e) -> p h e", e=65)[:, :, 64:65], 1.0)
        for tt in range(4):
            for hf in range(2):
                emit_v_unit((tt, hf))

        # ---- attention stages with injected proj work ----
        class _Stage:
            def __init__(self, p, ir, inject):
                self.p, self.ir = p, ir
                self.i0 = ir * 512
                self.jmax = 4 * (ir + 1)
                self.kpt = kpT[p]
                self.s_all, self.e_all = {}, {}
                self.inject = inject

            def scores(self, jb):
                c0 = max(0, jb - 4 * self.ir) * 128
                st = ps_gen.tile([128, 1024], F32, tag="ps_g", name="ps_g")
                for hh, pb in enumerate((0, 64)):
                    pe_chain(nc.tensor.matmul(
                        st[:, hh * 512 + c0:hh * 512 + 512],
                        self.kt[pb:pb + 64, jb * 128:(jb + 1) * 128],
                        self.qt[pb:pb + 64, self.i0 + c0:self.i0 + 512],
                        start=True, stop=True))
                self.s_all[jb] = st

            def exps(self, jb):
                c0 = max(0, jb - 4 * self.ir) * 128
                st = self.s_all.pop(jb)
                et = pexp.tile([128, 1024], BF, tag="et", name="et", bufs=6)
                nc.scalar.activation(
                    et.rearrange("p (g n) -> p g n", g=2)[:, :, c0:512],
                    st.rearrange("p (g n) -> p g n", g=2)[:, :, c0:512],
                    Exp, scale=0.125)
                if jb >= 4 * self.ir:
                    dv = et.rearrange("p (g n) -> p g n", g=2)[:, :,
                                                              c0:c0 + 128]
                    nc.gpsimd.tensor_mul(
                        dv, dv,
                        bass.AP(tensor=masksb.tensor,
                                offset=masksb.offset,
                                ap=[masksb.ap[0], [0, 2], masksb.ap[1]]))
                self.e_all[jb] = et

            def avs(self, jb):
                c0 = max(0, jb - 4 * self.ir) * 128
                et = self.e_all.pop(jb)
                for hh, acc in enumerate(self.Ats):
                    h = 2 * self.p + hh
                    pe_chain(nc.tensor.matmul(
                        acc[:, c0:512],
                        vsb[jb][:, h * 65:(h + 1) * 65],
                        et[:, hh * 512 + c0:hh * 512 + 512],
                        start=(jb == 0), stop=(jb == self.jmax - 1),
                        skip_group_check=True))

            def front1(self):
                spt = ps_gen.tile([64, 1024], F32, tag="ps_g", name="ps_g")
                for hh, pb in enumerate((0, 64)):
                    pe_chain(nc.tensor.matmul(
                        spt[:, hh * 512:hh * 512 + 512],
                        self.kpt[pb:pb + 64, :],
                        self.qt[pb:pb + 64, self.i0:self.i0 + 512],
                        start=True, stop=True))
                self.scores(0)
                ep = pexp.tile([64, 1024], BF, tag="ep", name="ep", bufs=2)
                nc.scalar.activation(ep, spt, Exp, scale=0.125)
                if self.ir == 0:
                    nc.gpsimd.tensor_mul(
                        ep.rearrange("p (g n) -> p g n", g=2)[:, :, 0:64],
                        ep.rearrange("p (g n) -> p g n", g=2)[:, :, 0:64],
                        bass.AP(tensor=masksb.tensor,
                                offset=masksb.offset,
                                ap=[[masksb.ap[0][0], 64], [0, 2],
                                    [masksb.ap[1][0], 64]]))
                self.eps = ep
                self.exps(0)

            @property
            def qt(self):
                return qkT_q[self.p]

            @property
            def kt(self):
                return qkT_k[self.p]

            def score_unit(self, jb):
                self.scores(jb)
                self.exps(jb)

            def av_front(self):
                self.Bts = [ps_acc.tile([65, 512], F32, tag="ps_a",
                                        name="ps_a") for _ in range(2)]
                for hh in range(2):
                    h = 2 * self.p + hh
                    pe_chain(nc.tensor.matmul(
                        self.Bts[hh], vpsb[0:64, h * 65:(h + 1) * 65],
                        self.eps[:, hh * 512:hh * 512 + 512],
                        start=True, stop=True))
                # extract prefix sums/y immediately: frees both PSUM slots
                for hh in range(2):
                    pb = hh * 64
                    nc.vector.tensor_copy(
                        sums_p[self.p][64 + 32 * hh:65 + 32 * hh,
                                       self.i0:self.i0 + 512],
                        self.Bts[hh][64:65, :])
                    nc.vector.tensor_copy(
                        yTb[self.p][pb:pb + 64, self.i0:self.i0 + 512],
                        self.Bts[hh][0:64, :])

            def av_unit(self, jb):
                if jb == 0:
                    self.Ats = [ps_acc.tile([65, 512], F32, tag="ps_a",
                                            name="ps_a") for _ in range(2)]
                self.avs(jb)

            def extract(self):
                for hh in range(2):
                    pb = hh * 64
                    nc.vector.tensor_copy(
                        sums_p[self.p][32 * hh:32 * hh + 1,
                                       self.i0:self.i0 + 512],
                        self.Ats[hh][64:65, :])
                    nc.vector.tensor_copy(
                        yTa[self.p][pb:pb + 64, self.i0:self.i0 + 512],
                        self.Ats[hh][0:64, :])

        # per-pair sum rows [a_h0, a_h1, b_h0, b_h1] and y accumulators;
        # combine for pair p runs at pair p+2, so 3 pairs stay live
        sums_p = [work.tile([128, T], F32, tag="sums", name=f"sums{p}", bufs=3)
                  for p in range(8)]
        # the three physical rotation buffers: init all rows to 1.0 so the
        # unused partitions stay recip-safe forever (only rows 0/32/64/96
        # are ever written)
        for pp in range(3):
            nc.vector.memset(sums_p[pp], 1.0)
        yTa = [None] * 8
        yTb = [None] * 8

        def alloc_pair_y(p):
            yTa[p] = work.tile([128, T], BF, tag="yTa", name=f"yTa{p}", bufs=3)
            yTb[p] = work.tile([128, T], BF, tag="yTb", name=f"yTb{p}", bufs=3)

        def emit_pair_combine(p):
            rec_f32 = work.tile([128, T], F32, tag="rec32", name="rec32", bufs=2)
            recips = work.tile([128, T], BF, tag="recbf", name="recbf", bufs=2)
            nc.vector.reciprocal_approx_fast(rec_f32, sums_p[p])
            nc.vector.tensor_copy(recips, rec_f32)
            for hf in range(2):
                s = slice(hf * 512, (hf + 1) * 512)
                bca = ps_acc.tile([128, 512], F32, tag="ps_a", name="ps_a")
                pe_chain(nc.tensor.matmul(bca, fmat4[:, 0:128], recips[:, s],
                                          start=True, stop=True))
                bcb = ps_acc.tile([128, 512], F32, tag="ps_a", name="ps_a")
                pe_chain(nc.tensor.matmul(bcb, fmat4[:, 128:256], recips[:, s],
                                          start=True, stop=True))
                bca_sb = pexp.tile([128, 512], BF, tag="bcs", name="bcs", bufs=4)
                bcb_sb = pexp.tile([128, 512], BF, tag="bcs", name="bcs", bufs=4)
                nc.vector.tensor_copy(bca_sb, bca)
                nc.vector.tensor_copy(bcb_sb, bcb)
                tmp = pexp.tile([128, 512], BF, tag="ctmp", name="ctmp", bufs=2)
                nc.gpsimd.tensor_mul(yT[p][:, s], yTa[p][:, s], bca_sb)
                nc.gpsimd.tensor_mul(tmp, yTb[p][:, s], bcb_sb)
                nc.gpsimd.tensor_add(yT[p][:, s], yT[p][:, s], tmp)

        # (pair_tag, fn, u): v-units tagged -1 (ps_gen only, safe to
        # force-drain anywhere); proj units tagged by their pair (ps_acc po
        # allocations -- at most 2 may inject per av-stage, see rotation
        # audit in the docstring)
        pending = []
        proj_pops = [0]

        def inject():
            # free v-units first
            while pending and pending[0][0] < 0:
                _, fn, u = pending.pop(0)
                fn(u)
            if pending and proj_pops[0] < 2:
                _, fn, u = pending.pop(0)
                fn(u)
                proj_pops[0] += 1

        def drain_v():
            while pending and pending[0][0] < 0:
                _, fn, u = pending.pop(0)
                fn(u)

        pending.extend([(-1, emit_v_unit, (tt, hf))
                        for tt in range(4, 8) for hf in range(2)])
        for pp in (1, 2, 3):
            for fn, u in emit_qk_proj(pp, dma_wqk_pair(pp)):
                pending.append((pp, fn, u))

        stages = []
        for p in range(8):
            for ir in range(2):
                stages.append(_Stage(p, ir, inject))

        score_q = []
        av_q = []
        for si, st in enumerate(stages):
            score_q.append((si, 'front1', st, None))
            for jb in range(1, st.jmax):
                score_q.append((si, 'score', st, jb))
            av_q.append((si, 'avfront', st, None))
            for jb in range(st.jmax):
                av_q.append((si, 'av', st, jb))
            av_q.append((si, 'extract', st, None))

        # zipper: the score+exp stream runs LAG av-consumable units ahead of
        # the AV stream, so no PE matmul ever waits on a fresh exp
        LAG = 4
        sc_n = avc_n = 0
        while score_q or av_q:
            if score_q and (sc_n - avc_n < LAG or not av_q):
                si, kind, st, jb = score_q.pop(0)
                if kind == 'front1' and st.ir == 0:
                    bad = [u for u in pending if 0 <= u[0] <= st.p]
                    assert not bad, f"proj units for pair {st.p} not drained"
                if kind == 'front1':
                    st.front1()
                else:
                    st.score_unit(jb)
                sc_n += 1
            else:
                si, kind, st, jb = av_q.pop(0)
                if kind == 'avfront':
                    proj_pops[0] = 0
                    if si == 1:
                        drain_v()
                    if st.ir == 0:
                        alloc_pair_y(st.p)
                        if st.p == 3:
                            for k in range(KC):
                                nc.sync.dma_start(
                                    out=wp_t[k],
                                    in_=dram["wp"].ap()[k * 128:(k + 1) * 128, :])
                        if st.p + 3 <= 7:
                            for fn, u in emit_qk_proj(
                                    st.p + 3, dma_wqk_pair(st.p + 3)):
                                pending.append((st.p + 3, fn, u))
                    st.av_front()
                elif kind == 'av':
                    st.av_unit(jb)
                    avc_n += 1
                    if avc_n % 2 == 0:
                        inject()
                else:
                    st.extract()
                    # post-extract is rotation-safe for po: Ats just died
                    proj_pops[0] = 0
                    inject()
                    inject()
                    if st.ir == 1 and 2 <= st.p:
                        emit_pair_combine(st.p - 2)
        while pending:
            _, fn, u = pending.pop(0)
            fn(u)
        lead_ctx.close()
        warm2 = ps_acc.tile([128, 128], F32, tag="ps_a", name="warm2")
        for _ in range(24):
            pe_chain(nc.tensor.matmul(warm2, masksb, masksb,
                                      start=True, stop=True))
        emit_pair_combine(6)
        emit_pair_combine(7)

        # ---- output projection: outT = wp-chunks.T @ yT ----
        pstg = top.enter_context(tc.tile_pool(name="pstg", bufs=4))
        for hf in range(2):
            for m in range(8):
                po = ps_acc.tile([128, 512], F32, tag="ps_a", name="po")
                for k in range(KC):
                    pe_chain(nc.tensor.matmul(
                        po, wp_t[k][:, m * 128:(m + 1) * 128],
                        yT[k][:, hf * 512:(hf + 1) * 512],
                        start=(k == 0), stop=(k == KC - 1)))
                stg = pstg.tile([128, 512], BF, tag="stg", name="stg")
                if m % 2 == 0:
                    nc.scalar.copy(stg, po)
                else:
                    nc.vector.tensor_copy(stg, po)
                blk = hf * 8 + m
                nc.sync.dma_start(
                    out=dram["out"].ap()[blk * 128:(blk + 1) * 128, :],
                    in_=stg)


def _build():
    if "nc" in _CACHE:
        return _CACHE["nc"]
    import concourse.mybir as mybir
    import concourse.tile as tile
    from concourse import bacc

    BF = mybir.dt.bfloat16
    nc = bacc.Bacc("TRN2", target_bir_lowering=False, debug=False,
                   enable_asserts=False)
    dram = {
        "xT": nc.dram_tensor("xT", [C, T], BF, kind="ExternalInput"),
        "pT": nc.dram_tensor("pT", [C, TP], BF, kind="ExternalInput"),
        "wqkp": nc.dram_tensor("wqkp", [8 * C, 256], BF, kind="ExternalInput"),
        "wv": nc.dram_tensor("wv", [C, C], BF, kind="ExternalInput"),
        "wkp": nc.dram_tensor("wkp", [C, C], BF, kind="ExternalInput"),
        "wvp": nc.dram_tensor("wvp", [C, C], BF, kind="ExternalInput"),
        "wp": nc.dram_tensor("wp", [C, C], BF, kind="ExternalInput"),
        "mask": nc.dram_tensor("mask", [128, 128], BF, kind="ExternalInput"),
        "maskp": nc.dram_tensor("maskp", [128, 64], BF, kind="ExternalInput"),
        "fmat4": nc.dram_tensor("fmat4", [128, 256], BF, kind="ExternalInput"),
        "out": nc.dram_tensor("out", [16 * 128, 512], BF, kind="ExternalOutput"),
    }
    with tile.TileContext(nc) as tc:
        _emit(nc, tc, dram)
    nc.compile()
    _CACHE["nc"] = nc
    return nc


def _host_consts():
    bf = ml_dtypes.bfloat16
    mask = np.triu(np.ones((128, 128), np.float32)).astype(bf)  # [p,f]=1 if f>=p
    tri = np.triu(np.ones((64, 64), np.float32))
    maskp = np.concatenate([tri, tri], axis=0).astype(bf)  # [128, 64]
    # fmat4[r, sel*128+p] = 1 iff r == sel*64 + (p>=64)*32; selects the
    # recip row for partition p of the A (sel=0) / B (sel=1) accumulator;
    # sum rows live at partitions 0/32/64/96
    fmat4 = np.zeros((128, 256), np.float32)
    for sel in range(2):
        for pp in range(128):
            fmat4[sel * 64 + (32 if pp >= 64 else 0), sel * 128 + pp] = 1.0
    return mask, maskp, fmat4.astype(bf)


def _make_in_maps(x, prefix_embd, w_attn, w_prefix, w_proj):
    bf = ml_dtypes.bfloat16
    x = np.asarray(x, np.float32)
    prefix_embd = np.asarray(prefix_embd, np.float32)
    w_attn = np.asarray(w_attn, np.float32)
    w_prefix = np.asarray(w_prefix, np.float32)
    w_proj = np.asarray(w_proj, np.float32)
    mask, maskp, fmat4 = _host_consts()
    wq = w_attn[:, 0:C]
    wk = w_attn[:, C:2 * C]
    # pair-major q/k weight: wqkp[p*C:(p+1)*C, 0:128]=wq cols, [.,128:256]=wk
    wqkp = np.zeros((8 * C, 256), np.float32)
    for p in range(8):
        wqkp[p * C:(p + 1) * C, 0:128] = wq[:, p * 128:(p + 1) * 128]
        wqkp[p * C:(p + 1) * C, 128:256] = wk[:, p * 128:(p + 1) * 128]
    wqkp = wqkp.astype(bf)
    wv = np.ascontiguousarray(w_attn[:, 2 * C:]).astype(bf)
    wkp = np.ascontiguousarray(w_prefix[:, C:2 * C]).astype(bf)
    wvp = np.ascontiguousarray(w_prefix[:, 2 * C:]).astype(bf)
    wp = w_proj.astype(bf)
    in_maps = []
    for i in range(B):
        in_maps.append({
            "xT": np.ascontiguousarray(x[i].T).astype(bf),
            "pT": np.ascontiguousarray(prefix_embd[i].T).astype(bf),
            "wqkp": wqkp, "wv": wv, "wkp": wkp, "wvp": wvp, "wp": wp,
            "mask": mask, "maskp": maskp, "fmat4": fmat4,
        })
    return in_maps


def kernel(x, prefix_embd, w_attn, b_attn, w_prefix, b_prefix, w_proj, b_proj,
           **_ignored):
    nc = _build()
    in_maps = _make_in_maps(x, prefix_embd, w_attn, w_prefix, w_proj)
    from concourse.bass_utils import run_bass_kernel_spmd
    res = run_bass_kernel_spmd(nc, in_maps, core_ids=list(range(B)))
    outs = []
    for i in range(B):
        blk = np.asarray(res.results[i]["out"], np.float32)
        blk = blk.reshape(2, 8, 128, 512)          # (hf, m, p, i)
        outT = blk.transpose(1, 2, 0, 3).reshape(C, T)   # [m*128+p, hf*512+i]
        outs.append(outT.T)
    return np.ascontiguousarray(np.stack(outs))
